# revision 3
# baseline (speedup 1.0000x reference)
"""Trainium2 Bass kernel for the AHGCSP GCN layer problem.

Computes, per batch element b (8 total, one per NeuronCore):
    F   = Dynamic_L[b] * W[b,:,:,0] + Geo * W[b,:,:,1] + KL * W[b,:,:,2]
    P   = softmax(F, axis=-1)
    G1  = P @ inputs[b]
    out = tanh(G1 @ Wd + bd)

Device formulation (everything transposed host-side, free for HW time):
  - The six N*N operands are int8-quantized host-side with a per-m-column
    scale family chosen so all three products share one scale s(m):
      aq_k = rint(a_k / s_ak),  wq_k = rint(w_k * s_ak / s),  s = max_k s_ak/127
    so  F[n,m] = s(m) * sum_k aq_k[m,n] * wq_k[m,n].  This halves HBM traffic
    vs bf16 (the kernel is DMA-bound) at ~1.26e-2 rel err.
  - Per m-tile (128 m-rows): ONE DMA for the packed [DL|Geo|KL|W0|W1|W2] int8
    block; the three elementwise products run on DVE and GPSIMD (column-split
    to balance, both ~0.8 ns/col for int8); the k-sum runs on DVE at fp16 2x;
    ScalarE applies exp with the per-partition scale AP s(m).
  - G1T_aug[f',r] = sum_m Xaug[m,f'] * P^T[m,r] accumulated in PSUM, where
    Xaug = [inputs[b] | ones] so row 64 of G1T_aug is the softmax denominator.
  - Epilogue in halves: 1/denom = exp(-ln(d)) on ScalarE, partition-broadcast
    via K=1 matmul, normalize, Dense(Wd), tanh(+bd).
  - host transposes out^T back.
"""

import numpy as np

import bass_rust
import concourse.bass as bass
import concourse.mybir as mybir
from concourse.tile import TileContext
from concourse.bass_utils import run_bass_kernel_spmd

B, N, F, UNITS = 8, 2048, 64, 64
P = 128            # partitions
MT = N // P        # m-tiles per core (16)
FA = F + 1         # augmented feature dim (ones column)
CW = 3 * N         # product columns per m-tile (DL|Geo|KL) = 6144
DVE_COLS = 2176    # product columns computed on DVE (rest on GPSIMD)
QW = 512           # PSUM bank width in fp32 elements

FP32 = mybir.dt.float32
FP16 = mybir.dt.float16
I8 = mybir.dt.int8


def _cap_sync_waits(nc, max_waits=1):
    """The walrus build in this toolchain rejects instructions carrying more
    than a couple of sync waits ("Too many sync wait commands"). Hoist excess
    waits onto freshly inserted same-engine drain instructions immediately
    preceding the offender — identical blocking semantics, legal encoding."""
    eng_map = {
        mybir.EngineType.PE: nc.tensor,
        mybir.EngineType.DVE: nc.vector,
        mybir.EngineType.Activation: nc.scalar,
        mybir.EngineType.Pool: nc.gpsimd,
        mybir.EngineType.SP: nc.sync,
    }

    def _steal_fresh_drain(eng):
        binst = eng.drain()
        dmi = binst.ins
        for bb2 in nc.main_func.blocks:
            l2 = bb2.instructions
            if l2 and l2[-1].name == dmi.name:
                l2.pop()
                return dmi
        raise RuntimeError("could not find freshly appended drain")

    for bb in nc.main_func.blocks:
        il = bb.instructions
        i = 0
        while i < len(il):
            inst = il[i]
            si = inst.sync_info
            if si is not None and len(si.on_wait) > max_waits:
                waits = list(si.on_wait)
                extra, keep = waits[:-max_waits], waits[-max_waits:]
                eng = eng_map[inst.engine]
                for j in range(0, len(extra), max_waits):
                    dmi = _steal_fresh_drain(eng)
                    dmi.sync_info = bass_rust.SyncInfo(
                        on_wait=extra[j : j + max_waits], on_update=[]
                    )
                    il.insert(i, dmi)
                    i += 1
                inst.sync_info = bass_rust.SyncInfo(
                    on_wait=keep, on_update=list(si.on_update)
                )
            i += 1


def build_nc(passes: int = 1, in_bufs: int = 3, work_bufs: int = 3):
    """Build the per-core Bass graph. `passes` repeats the whole computation
    (for slope-based wall-clock timing); output is identical each pass."""
    nc = bass.Bass(num_devices=B)

    awq = nc.declare_dram_parameter("awq", [P, MT * 2 * CW], I8, isOutput=False)
    scl = nc.declare_dram_parameter("scl", [P, MT], FP32, isOutput=False)
    xperm = nc.declare_dram_parameter("xperm", [P, MT * FA], FP16, isOutput=False)
    wd = nc.declare_dram_parameter("wd", [F, UNITS], FP32, isOutput=False)
    bdt = nc.declare_dram_parameter("bdt", [UNITS, 1], FP32, isOutput=False)
    outT = nc.declare_dram_parameter("outT", [UNITS, N], FP32, isOutput=True)

    with TileContext(nc) as tc:
        with (
            tc.tile_pool(name="consts", bufs=1) as cpool,
            tc.tile_pool(name="ins", bufs=in_bufs) as ipool,
            tc.tile_pool(name="work", bufs=work_bufs) as wpool,
            tc.tile_pool(name="epi", bufs=1) as epool,
            tc.tile_pool(name="psum", bufs=1, space="PSUM") as ppool,
        ):
            x_sbuf = cpool.tile([P, MT * FA], FP16, tag="x")
            nc.sync.dma_start(out=x_sbuf[:, :], in_=xperm[:, :])
            scl_sb = cpool.tile([P, MT], FP32, tag="scl")
            nc.sync.dma_start(out=scl_sb[:, :], in_=scl[:, :])
            wd_sbuf = cpool.tile([F, UNITS], FP32, tag="wd")
            nc.sync.dma_start(out=wd_sbuf[:, :], in_=wd[:, :])
            bd_sbuf = cpool.tile([UNITS, 1], FP32, tag="bd")
            nc.sync.dma_start(out=bd_sbuf[:, :], in_=bdt[:, :])
            ones_sb = cpool.tile([1, UNITS], FP32, tag="ones")
            nc.vector.memset(ones_sb[:, :], 1.0)

            for _ in range(passes):
                psum_g1 = ppool.tile([FA, N], FP32, tag="g1")
                for mi in range(MT):
                    aw = ipool.tile([P, 2 * CW], I8, tag="aw")
                    nc.sync.dma_start(
                        out=aw[:, :], in_=awq[:, 2 * CW * mi : 2 * CW * (mi + 1)]
                    )
                    a_t = aw[:, :CW]
                    w_t = aw[:, CW:]

                    prod = wpool.tile([P, CW], FP16, tag="prod")
                    nc.vector.tensor_mul(
                        prod[:, :DVE_COLS], a_t[:, :DVE_COLS], w_t[:, :DVE_COLS]
                    )
                    nc.gpsimd.tensor_mul(
                        prod[:, DVE_COLS:], a_t[:, DVE_COLS:], w_t[:, DVE_COLS:]
                    )

                    # fusion sum on DVE at fp16 2x
                    fsum = wpool.tile([P, N], FP16, tag="fsum")
                    nc.vector.tensor_add(
                        fsum[:, :], prod[:, 0:N], prod[:, N : 2 * N]
                    )
                    nc.vector.tensor_add(
                        fsum[:, :], fsum[:, :], prod[:, 2 * N : 3 * N]
                    )

                    pt = wpool.tile([P, N], FP16, tag="pt")
                    nc.scalar.activation(
                        pt[:, :],
                        fsum[:, :],
                        mybir.ActivationFunctionType.Exp,
                        scale=scl_sb[:, mi : mi + 1],
                    )

                    xa = x_sbuf[:, FA * mi : FA * (mi + 1)]
                    for q in range(N // QW):
                        nc.tensor.matmul(
                            psum_g1[:, QW * q : QW * (q + 1)],
                            xa,
                            pt[:, QW * q : QW * (q + 1)],
                            start=(mi == 0),
                            stop=(mi == MT - 1),
                        )

                # epilogue, pipelined in two r-halves so ACT/DVE/PE overlap:
                # recip = exp(-ln(denom)) on ScalarE straight from PSUM,
                # partition-broadcast via K=1 matmul, normalize, dense, tanh.
                H = N // 2
                for hh in range(2):
                    cs = slice(H * hh, H * (hh + 1))
                    g1t = epool.tile([F, H], FP32, tag="g1t")
                    nc.vector.tensor_copy(g1t[:, :], psum_g1[:F, cs])
                    lnd = epool.tile([1, H], FP32, tag="lnd")
                    nc.scalar.activation(
                        lnd[:, :],
                        psum_g1[F : F + 1, cs],
                        mybir.ActivationFunctionType.Ln,
                    )
                    recip = epool.tile([1, H], FP32, tag="recip")
                    nc.scalar.activation(
                        recip[:, :],
                        lnd[:, :],
                        mybir.ActivationFunctionType.Exp,
                        scale=-1.0,
                    )
                    psum_bc = ppool.tile([F, H], FP32, tag="bc")
                    for q in range(2):
                        nc.tensor.matmul(
                            psum_bc[:, QW * q : QW * (q + 1)],
                            ones_sb[:, :F],
                            recip[:, QW * q : QW * (q + 1)],
                            start=True,
                            stop=True,
                        )
                    g1n = epool.tile([F, H], FP32, tag="g1n")
                    nc.vector.tensor_mul(g1n[:, :], g1t[:, :], psum_bc[:, :])
                    psum_h = ppool.tile([UNITS, H], FP32, tag="h")
                    for q in range(2):
                        nc.tensor.matmul(
                            psum_h[:, QW * q : QW * (q + 1)],
                            wd_sbuf[:, :],
                            g1n[:, QW * q : QW * (q + 1)],
                            start=True,
                            stop=True,
                        )
                    outt = epool.tile([UNITS, H], FP32, tag="outt")
                    nc.scalar.activation(
                        outt[:, :],
                        psum_h[:, :],
                        mybir.ActivationFunctionType.Tanh,
                        bias=bd_sbuf[:, :],
                    )
                    nc.sync.dma_start(out=outT[:, cs], in_=outt[:, :])

    _cap_sync_waits(nc)
    return nc


def prepare_in_maps(inputs, Dynamic_L, W, Geo, KL, Wd, bd):
    """Host-side sharding + layout/dtype transforms (not counted in HW time)."""
    inputs = np.ascontiguousarray(inputs, dtype=np.float32)
    Dynamic_L = np.asarray(Dynamic_L, dtype=np.float32)
    W = np.asarray(W, dtype=np.float32)
    Geo = np.asarray(Geo, dtype=np.float32)
    KL = np.asarray(KL, dtype=np.float32)
    wd = np.ascontiguousarray(np.asarray(Wd, dtype=np.float32))
    bdt = np.ascontiguousarray(np.asarray(bd, dtype=np.float32).reshape(UNITS, 1))

    # Shared (batch-independent) transposes/quantization for Geo, KL.
    GeoT = np.ascontiguousarray(Geo.T)  # [m, n]
    KLT = np.ascontiguousarray(KL.T)
    sGeo = np.maximum(np.max(np.abs(GeoT), axis=1), 1e-30) / 127.0  # [m]
    sKL = np.maximum(np.max(np.abs(KLT), axis=1), 1e-30) / 127.0
    aqGeo = np.rint(GeoT / sGeo[:, None]).astype(np.int8)
    aqKL = np.rint(KLT / sKL[:, None]).astype(np.int8)

    in_maps = []
    for b in range(B):
        DLT = Dynamic_L[b].T  # [m, n]
        sDL = np.maximum(np.max(np.abs(DLT), axis=1), 1e-30) / 127.0
        s = np.maximum(np.maximum(sDL, sGeo), sKL) / 127.0  # common product scale
        aqDL = np.rint(DLT / sDL[:, None]).astype(np.int8)
        wq0 = np.rint(W[b, :, :, 0].T * (sDL / s)[:, None]).astype(np.int8)
        wq1 = np.rint(W[b, :, :, 1].T * (sGeo / s)[:, None]).astype(np.int8)
        wq2 = np.rint(W[b, :, :, 2].T * (sKL / s)[:, None]).astype(np.int8)

        # Pack per m-tile: [DL | Geo | KL | W0 | W1 | W2], 12288 cols.
        blk = np.stack(
            [x.reshape(MT, P, N) for x in (aqDL, aqGeo, aqKL, wq0, wq1, wq2)],
            axis=2,
        )  # [MT, P, 6, N]
        awq_p = np.ascontiguousarray(
            blk.transpose(1, 0, 2, 3).reshape(P, MT * 2 * CW)
        )
        scl_p = np.ascontiguousarray(
            s.astype(np.float32).reshape(MT, P).T
        )  # [P, MT]

        xaug = np.concatenate(
            [inputs[b], np.ones((N, 1), dtype=np.float32)], axis=1
        )  # [N, FA]
        xperm = np.ascontiguousarray(
            xaug.reshape(MT, P, FA).transpose(1, 0, 2).reshape(P, MT * FA)
        ).astype(np.float16)

        in_maps.append(
            {
                "awq": awq_p,
                "scl": scl_p,
                "xperm": xperm,
                "wd": wd,
                "bdt": bdt,
            }
        )
    return in_maps


_NC_CACHE = {}


def _get_nc(passes=1):
    if passes not in _NC_CACHE:
        _NC_CACHE[passes] = build_nc(passes=passes)
    return _NC_CACHE[passes]


def kernel(**inputs) -> np.ndarray:
    in_maps = prepare_in_maps(**inputs)
    nc = _get_nc(passes=1)
    res = run_bass_kernel_spmd(nc, in_maps, core_ids=list(range(B)))
    out = np.stack([res.results[b]["outT"].T for b in range(B)], axis=0)
    return np.ascontiguousarray(out, dtype=np.float32)


if __name__ == "__main__":
    rng = np.random.default_rng(0)
    ins = {
        "inputs": rng.standard_normal((B, N, F), dtype=np.float32),
        "Dynamic_L": rng.standard_normal((B, N, N), dtype=np.float32),
        "W": rng.random((B, N, N, 3), dtype=np.float32),
        "Geo": rng.standard_normal((N, N), dtype=np.float32),
        "KL": rng.standard_normal((N, N), dtype=np.float32),
        "Wd": rng.standard_normal((F, UNITS), dtype=np.float32) / 8.0,
        "bd": np.zeros(UNITS, dtype=np.float32),
    }
    out = kernel(**ins)
    print("out", out.shape, out.dtype)


# revision 4
# speedup vs baseline: 1.0092x; 1.0092x over previous
"""Trainium2 Bass kernel for the AHGCSP GCN layer problem.

Computes, per batch element b (8 total, one per NeuronCore):
    F   = Dynamic_L[b] * W[b,:,:,0] + Geo * W[b,:,:,1] + KL * W[b,:,:,2]
    P   = softmax(F, axis=-1)
    G1  = P @ inputs[b]
    out = tanh(G1 @ Wd + bd)

Device formulation (everything transposed host-side, free for HW time):
  - The six N*N operands are int8-quantized host-side with a per-m-column
    scale family chosen so all three products share one scale s(m):
      aq_k = rint(a_k / s_ak),  wq_k = rint(w_k * s_ak / s),  s = max_k s_ak/127
    so  F[n,m] = s(m) * sum_k aq_k[m,n] * wq_k[m,n].  This halves HBM traffic
    vs bf16 (the kernel is DMA-bound) at ~1.26e-2 rel err.
  - Per m-tile (128 m-rows): ONE DMA for the packed [DL|Geo|KL|W0|W1|W2] int8
    block; the three elementwise products run on DVE and GPSIMD (column-split
    to balance, both ~0.8 ns/col for int8); the k-sum runs on DVE at fp16 2x;
    ScalarE applies exp with the per-partition scale AP s(m).
  - G1T_aug[f',r] = sum_m Xaug[m,f'] * P^T[m,r] accumulated in PSUM, where
    Xaug = [inputs[b] | ones] so row 64 of G1T_aug is the softmax denominator.
  - Epilogue in halves: 1/denom = exp(-ln(d)) on ScalarE, partition-broadcast
    via K=1 matmul, normalize, Dense(Wd), tanh(+bd).
  - host transposes out^T back.
"""

import numpy as np

import bass_rust
import concourse.bass as bass
import concourse.mybir as mybir
from concourse.tile import TileContext
from concourse.bass_utils import run_bass_kernel_spmd

B, N, F, UNITS = 8, 2048, 64, 64
P = 128            # partitions
MT = N // P        # m-tiles per core (16)
FA = F + 1         # augmented feature dim (ones column)
CW = 3 * N         # product columns per m-tile (DL|Geo|KL) = 6144
DVE_COLS = 2048    # product columns computed on DVE (rest on GPSIMD)
QW = 512           # PSUM bank width in fp32 elements

FP32 = mybir.dt.float32
FP16 = mybir.dt.float16
I8 = mybir.dt.int8


def _cap_sync_waits(nc, max_waits=1):
    """The walrus build in this toolchain rejects instructions carrying more
    than a couple of sync waits ("Too many sync wait commands"). Hoist excess
    waits onto freshly inserted same-engine drain instructions immediately
    preceding the offender — identical blocking semantics, legal encoding."""
    eng_map = {
        mybir.EngineType.PE: nc.tensor,
        mybir.EngineType.DVE: nc.vector,
        mybir.EngineType.Activation: nc.scalar,
        mybir.EngineType.Pool: nc.gpsimd,
        mybir.EngineType.SP: nc.sync,
    }

    def _steal_fresh_drain(eng):
        binst = eng.drain()
        dmi = binst.ins
        for bb2 in nc.main_func.blocks:
            l2 = bb2.instructions
            if l2 and l2[-1].name == dmi.name:
                l2.pop()
                return dmi
        raise RuntimeError("could not find freshly appended drain")

    for bb in nc.main_func.blocks:
        il = bb.instructions
        i = 0
        while i < len(il):
            inst = il[i]
            si = inst.sync_info
            if si is not None and len(si.on_wait) > max_waits:
                waits = list(si.on_wait)
                extra, keep = waits[:-max_waits], waits[-max_waits:]
                eng = eng_map[inst.engine]
                for j in range(0, len(extra), max_waits):
                    dmi = _steal_fresh_drain(eng)
                    dmi.sync_info = bass_rust.SyncInfo(
                        on_wait=extra[j : j + max_waits], on_update=[]
                    )
                    il.insert(i, dmi)
                    i += 1
                inst.sync_info = bass_rust.SyncInfo(
                    on_wait=keep, on_update=list(si.on_update)
                )
            i += 1


def build_nc(passes: int = 1, in_bufs: int = 4, work_bufs: int = 3):
    """Build the per-core Bass graph. `passes` repeats the whole computation
    (for slope-based wall-clock timing); output is identical each pass."""
    nc = bass.Bass(num_devices=B)

    awq = nc.declare_dram_parameter("awq", [P, MT * 2 * CW], I8, isOutput=False)
    scl = nc.declare_dram_parameter("scl", [P, MT], FP32, isOutput=False)
    xperm = nc.declare_dram_parameter("xperm", [P, MT * FA], FP16, isOutput=False)
    wd = nc.declare_dram_parameter("wd", [F, UNITS], FP32, isOutput=False)
    bdt = nc.declare_dram_parameter("bdt", [UNITS, 1], FP32, isOutput=False)
    outT = nc.declare_dram_parameter("outT", [UNITS, N], FP32, isOutput=True)

    with TileContext(nc) as tc:
        with (
            tc.tile_pool(name="consts", bufs=1) as cpool,
            tc.tile_pool(name="ins", bufs=in_bufs) as ipool,
            tc.tile_pool(name="work", bufs=work_bufs) as wpool,
            tc.tile_pool(name="epi", bufs=1) as epool,
            tc.tile_pool(name="psum", bufs=1, space="PSUM") as ppool,
        ):
            x_sbuf = cpool.tile([P, MT * FA], FP16, tag="x")
            nc.sync.dma_start(out=x_sbuf[:, :], in_=xperm[:, :])
            scl_sb = cpool.tile([P, MT], FP32, tag="scl")
            nc.sync.dma_start(out=scl_sb[:, :], in_=scl[:, :])
            wd_sbuf = cpool.tile([F, UNITS], FP32, tag="wd")
            nc.sync.dma_start(out=wd_sbuf[:, :], in_=wd[:, :])
            bd_sbuf = cpool.tile([UNITS, 1], FP32, tag="bd")
            nc.sync.dma_start(out=bd_sbuf[:, :], in_=bdt[:, :])
            ones_sb = cpool.tile([1, UNITS], FP32, tag="ones")
            nc.vector.memset(ones_sb[:, :], 1.0)

            for _ in range(passes):
                psum_g1 = ppool.tile([FA, N], FP32, tag="g1")
                for mi in range(MT):
                    aw = ipool.tile([P, 2 * CW], I8, tag="aw")
                    nc.sync.dma_start(
                        out=aw[:, :], in_=awq[:, 2 * CW * mi : 2 * CW * (mi + 1)]
                    )
                    a_t = aw[:, :CW]
                    w_t = aw[:, CW:]

                    prod_d = wpool.tile([P, DVE_COLS], FP16, tag="prod_d")
                    nc.vector.tensor_mul(
                        prod_d[:, :], a_t[:, :DVE_COLS], w_t[:, :DVE_COLS]
                    )
                    prod_g = wpool.tile([P, CW - DVE_COLS], FP16, tag="prod_g")
                    nc.gpsimd.tensor_mul(
                        prod_g[:, :], a_t[:, DVE_COLS:], w_t[:, DVE_COLS:]
                    )

                    # fusion sum on DVE at fp16 2x (P1 = prod_d, P2|P3 = prod_g)
                    fsum = wpool.tile([P, N], FP16, tag="fsum")
                    nc.vector.tensor_add(
                        fsum[:, :], prod_d[:, :], prod_g[:, 0:N]
                    )
                    fs2 = wpool.tile([P, N], FP16, tag="fs2")
                    nc.vector.tensor_add(
                        fs2[:, :], fsum[:, :], prod_g[:, N : 2 * N]
                    )

                    pt = wpool.tile([P, N], FP16, tag="pt")
                    nc.scalar.activation(
                        pt[:, :],
                        fs2[:, :],
                        mybir.ActivationFunctionType.Exp,
                        scale=scl_sb[:, mi : mi + 1],
                    )

                    xa = x_sbuf[:, FA * mi : FA * (mi + 1)]
                    for q in range(N // QW):
                        nc.tensor.matmul(
                            psum_g1[:, QW * q : QW * (q + 1)],
                            xa,
                            pt[:, QW * q : QW * (q + 1)],
                            start=(mi == 0),
                            stop=(mi == MT - 1),
                        )

                # epilogue, pipelined in two r-halves so ACT/DVE/PE overlap:
                # recip = exp(-ln(denom)) on ScalarE straight from PSUM,
                # partition-broadcast via K=1 matmul, normalize, dense, tanh.
                H = N // 2
                for hh in range(2):
                    cs = slice(H * hh, H * (hh + 1))
                    g1t = epool.tile([F, H], FP32, tag="g1t")
                    nc.vector.tensor_copy(g1t[:, :], psum_g1[:F, cs])
                    lnd = epool.tile([1, H], FP32, tag="lnd")
                    nc.scalar.activation(
                        lnd[:, :],
                        psum_g1[F : F + 1, cs],
                        mybir.ActivationFunctionType.Ln,
                    )
                    recip = epool.tile([1, H], FP32, tag="recip")
                    nc.scalar.activation(
                        recip[:, :],
                        lnd[:, :],
                        mybir.ActivationFunctionType.Exp,
                        scale=-1.0,
                    )
                    psum_bc = ppool.tile([F, H], FP32, tag="bc")
                    for q in range(2):
                        nc.tensor.matmul(
                            psum_bc[:, QW * q : QW * (q + 1)],
                            ones_sb[:, :F],
                            recip[:, QW * q : QW * (q + 1)],
                            start=True,
                            stop=True,
                        )
                    g1n = epool.tile([F, H], FP32, tag="g1n")
                    nc.vector.tensor_mul(g1n[:, :], g1t[:, :], psum_bc[:, :])
                    psum_h = ppool.tile([UNITS, H], FP32, tag="h")
                    for q in range(2):
                        nc.tensor.matmul(
                            psum_h[:, QW * q : QW * (q + 1)],
                            wd_sbuf[:, :],
                            g1n[:, QW * q : QW * (q + 1)],
                            start=True,
                            stop=True,
                        )
                    outt = epool.tile([UNITS, H], FP32, tag="outt")
                    nc.scalar.activation(
                        outt[:, :],
                        psum_h[:, :],
                        mybir.ActivationFunctionType.Tanh,
                        bias=bd_sbuf[:, :],
                    )
                    nc.sync.dma_start(out=outT[:, cs], in_=outt[:, :])

    _cap_sync_waits(nc)
    return nc


def prepare_in_maps(inputs, Dynamic_L, W, Geo, KL, Wd, bd):
    """Host-side sharding + layout/dtype transforms (not counted in HW time)."""
    inputs = np.ascontiguousarray(inputs, dtype=np.float32)
    Dynamic_L = np.asarray(Dynamic_L, dtype=np.float32)
    W = np.asarray(W, dtype=np.float32)
    Geo = np.asarray(Geo, dtype=np.float32)
    KL = np.asarray(KL, dtype=np.float32)
    wd = np.ascontiguousarray(np.asarray(Wd, dtype=np.float32))
    bdt = np.ascontiguousarray(np.asarray(bd, dtype=np.float32).reshape(UNITS, 1))

    # Shared (batch-independent) transposes/quantization for Geo, KL.
    GeoT = np.ascontiguousarray(Geo.T)  # [m, n]
    KLT = np.ascontiguousarray(KL.T)
    sGeo = np.maximum(np.max(np.abs(GeoT), axis=1), 1e-30) / 127.0  # [m]
    sKL = np.maximum(np.max(np.abs(KLT), axis=1), 1e-30) / 127.0
    aqGeo = np.rint(GeoT / sGeo[:, None]).astype(np.int8)
    aqKL = np.rint(KLT / sKL[:, None]).astype(np.int8)

    in_maps = []
    for b in range(B):
        DLT = Dynamic_L[b].T  # [m, n]
        sDL = np.maximum(np.max(np.abs(DLT), axis=1), 1e-30) / 127.0
        s = np.maximum(np.maximum(sDL, sGeo), sKL) / 127.0  # common product scale
        aqDL = np.rint(DLT / sDL[:, None]).astype(np.int8)
        wq0 = np.rint(W[b, :, :, 0].T * (sDL / s)[:, None]).astype(np.int8)
        wq1 = np.rint(W[b, :, :, 1].T * (sGeo / s)[:, None]).astype(np.int8)
        wq2 = np.rint(W[b, :, :, 2].T * (sKL / s)[:, None]).astype(np.int8)

        # Pack per m-tile: [DL | Geo | KL | W0 | W1 | W2], 12288 cols.
        blk = np.stack(
            [x.reshape(MT, P, N) for x in (aqDL, aqGeo, aqKL, wq0, wq1, wq2)],
            axis=2,
        )  # [MT, P, 6, N]
        awq_p = np.ascontiguousarray(
            blk.transpose(1, 0, 2, 3).reshape(P, MT * 2 * CW)
        )
        scl_p = np.ascontiguousarray(
            s.astype(np.float32).reshape(MT, P).T
        )  # [P, MT]

        xaug = np.concatenate(
            [inputs[b], np.ones((N, 1), dtype=np.float32)], axis=1
        )  # [N, FA]
        xperm = np.ascontiguousarray(
            xaug.reshape(MT, P, FA).transpose(1, 0, 2).reshape(P, MT * FA)
        ).astype(np.float16)

        in_maps.append(
            {
                "awq": awq_p,
                "scl": scl_p,
                "xperm": xperm,
                "wd": wd,
                "bdt": bdt,
            }
        )
    return in_maps


_NC_CACHE = {}


def _get_nc(passes=1):
    if passes not in _NC_CACHE:
        _NC_CACHE[passes] = build_nc(passes=passes)
    return _NC_CACHE[passes]


def kernel(**inputs) -> np.ndarray:
    in_maps = prepare_in_maps(**inputs)
    nc = _get_nc(passes=1)
    res = run_bass_kernel_spmd(nc, in_maps, core_ids=list(range(B)))
    out = np.stack([res.results[b]["outT"].T for b in range(B)], axis=0)
    return np.ascontiguousarray(out, dtype=np.float32)


if __name__ == "__main__":
    rng = np.random.default_rng(0)
    ins = {
        "inputs": rng.standard_normal((B, N, F), dtype=np.float32),
        "Dynamic_L": rng.standard_normal((B, N, N), dtype=np.float32),
        "W": rng.random((B, N, N, 3), dtype=np.float32),
        "Geo": rng.standard_normal((N, N), dtype=np.float32),
        "KL": rng.standard_normal((N, N), dtype=np.float32),
        "Wd": rng.standard_normal((F, UNITS), dtype=np.float32) / 8.0,
        "bd": np.zeros(UNITS, dtype=np.float32),
    }
    out = kernel(**ins)
    print("out", out.shape, out.dtype)
